# revision 11
# baseline (speedup 1.0000x reference)
# Trainium2 Bass kernel for nn_Conv2dSDK_QR: low-rank (Q @ R) factorized
# stride-1 3x3 conv expressed as two matmuls over 4x4/stride-2 windows.
#
# Math (per image, validated vs reference):
#   xp = zero-pad(x, 1)                              [128, 66, 66]
#   flatT[win*128+c, vi*32+vj] = xp[c, i+2vi, j+2vj] (win = i*4+j)
#   tT = R2 @ flatT                                  [256, 1024]
#   yT = Q @ tT                                      [512, 1024]
#   out[oc, 2vi+top, 2vj+left] = yT[(top*2+left)*128+oc, vi*32+vj]
# where R2 is R with columns permuted from (c*16+win) to (win*128+c)
# ordering, so each win-chunk of flatT is just a strided view of xp.
#
# Device layouts (host pre/post-processed so every PE stream and every DMA
# is contiguous):
#   space-to-depth: s2d[c, pi, pj, hi, wi] = xp[c, 2hi+pi, 2wi+pj] (66=2x33)
#   x3[lb][c, pi, pj, h, w] = s2d[c, pi, pj, 16*lb + h, w], h in [0,17)
#     (l-block chunks, boundary row hi=16 duplicated) -> window (i,j) of
#     l-block lb is the contiguous-inner view
#     x3[lb][:, i&1, j&1, (i>>1):(i>>1)+16, (j>>1):(j>>1)+32]
#   y per l-block: y3[lb][oc, top, left, vi_in, vj] = out[oc, 2(16lb+vi_in)+top, 2vj+left]
#
# Sharding: data-parallel over batch, 4 images per core across 8 cores.
#
# Schedule notes (from ntff profile analysis):
# - The framework preamble ends ~7.3us; DMA packets start landing ~1.5us
#   after the first dma_start issue. The tensor engine runs at a reduced
#   p-state for its first ~3us of continuous execution, so a string of
#   warmup matmuls on a memset tile is issued first: they execute during
#   the input-DMA head and bring the PE to full clock before real work.
# - The sync ring carries everything the first two images' matmuls gate
#   on, interleaved finest-first: [r2t0 w0-3][x0 lb0 pi0][r2t0 w4-11]
#   [x0 lb0 pi1][r2t0 w12-15][x0 lb1][r2t1][x1]. The scalar ring carries
#   qt + images 2,3 + all output, so the slow gpsimd SWDGE ring is unused.
# - bf16 end-to-end (rel err ~3e-3, gate 2e-2): halves DMA bytes and DVE
#   copy traffic vs f32; matmul rate on the PE is 1 row/cycle either way.

import numpy as np

import concourse.bacc as bacc
import concourse.bass as bass
import concourse.mybir as mybir
import concourse.tile as tile
from concourse.bass_utils import run_bass_kernel_spmd

N_CORES = 8
N_PER_CORE = 4
C = 128          # channels (= partition dim)
H = W = 64
RANK = 256
MOUT = 512       # 4 placements * 128 out channels
NWIN = 16        # 4x4 window positions
DT = mybir.dt.float32
MM_DT = mybir.dt.bfloat16
NWARM = 12       # p-state warmup matmuls (N=256 each) before real work
# Window processing order: even-i (pi=0) windows first, so the first matmul
# group can start as soon as the pi=0 half of the image chunk and the first
# weight chunk have landed. The r2t win axis is host-permuted to this order.
WIN_SEQ = [0, 1, 2, 3, 8, 9, 10, 11, 4, 5, 6, 7, 12, 13, 14, 15]


def _out_dt(mm_dtype):
    # bf16 output halves the write traffic; keep f32 output for the f32r
    # variant so it stays the high-precision reference configuration.
    return mybir.dt.bfloat16 if mm_dtype == mybir.dt.bfloat16 else DT


def build_nc(n_per_core=N_PER_CORE, mm_dtype=MM_DT):
    out_dt = _out_dt(mm_dtype)
    nc = bacc.Bacc()
    x_ext = nc.declare_dram_parameter("x", [n_per_core, 2, C, 2, 2, 17, 33], mm_dtype, isOutput=False)
    r_ext = nc.declare_dram_parameter("r2t", [2, C, NWIN, 128], mm_dtype, isOutput=False)
    q_ext = nc.declare_dram_parameter("qt", [C, 2, MOUT], mm_dtype, isOutput=False)
    y_ext = nc.declare_dram_parameter("y", [n_per_core, 2, C, 2, 2, 16, 32], out_dt, isOutput=True)

    with tile.TileContext(nc) as tc:
        with (
            tc.tile_pool(name="weights", bufs=1) as wpool,
            tc.tile_pool(name="xp", bufs=8) as xpool,
            tc.tile_pool(name="tt", bufs=2) as tpool,
            tc.tile_pool(name="osb", bufs=4) as opool,
            tc.tile_pool(name="pt", bufs=4, space="PSUM") as ptpool,
            tc.tile_pool(name="py", bufs=4, space="PSUM") as pypool,
        ):
            # PE p-state warmup: matmuls on a zeroed tile, results discarded.
            # These run during the input-DMA head so the first real matmul
            # already executes at full clock.
            warm = wpool.tile([C, 256], mm_dtype, name="warm")
            nc.gpsimd.memset(warm[:], 0.0)
            wpt = ptpool.tile([128, 16, 32], DT, tag="pt", name="wpt")
            for _ in range(NWARM):
                nc.tensor.matmul(wpt[:, :8], warm[:, :128], warm[:], start=True, stop=True)

            # r2t[rc][c, win, r_in] = R2[rc*128+r_in, win*128+c]; the win axis
            # is host-permuted to WIN_SEQ order, so [:, :4] is the first four
            # windows processed.
            r2t = [wpool.tile([C, NWIN, 128], mm_dtype, tag=f"r2t{rc}", name=f"r2t{rc}") for rc in range(2)]
            # qt[r_in, rc, m] = Q[m, rc*128+r_in]  (lhsT chunks for matmul 2)
            qt = wpool.tile([C, 2, MOUT], mm_dtype)
            nc.sync.dma_start(r2t[0][:, :4], r_ext[0][:, :4])
            # qt is only needed by the first matmul-2 group (~16us after the
            # first matmul); the scalar HWDGE ring has it long before then.
            nc.scalar.dma_start(qt[:], q_ext[:])

            for n in range(n_per_core):
                x3 = [xpool.tile([C, 2, 2, 17, 33], mm_dtype, tag="x3", name=f"x3_{n}_{lb}") for lb in range(2)]
                if n == 0:
                    # Interleave weight chunks with the pi-plane halves the
                    # first matmuls consume, finest chunks first, all on the
                    # early-starting sync ring.
                    nc.sync.dma_start(x3[0][:, 0], x_ext[0, 0, :, 0])
                    nc.sync.dma_start(r2t[0][:, 4:12], r_ext[0][:, 4:12])
                    nc.sync.dma_start(x3[0][:, 1], x_ext[0, 0, :, 1])
                    nc.sync.dma_start(r2t[0][:, 12:], r_ext[0][:, 12:])
                    nc.sync.dma_start(x3[1][:, 0], x_ext[0, 1, :, 0])
                    nc.sync.dma_start(x3[1][:, 1], x_ext[0, 1, :, 1])
                    nc.sync.dma_start(r2t[1][:], r_ext[1])
                elif n == 1:
                    for lb in range(2):
                        nc.sync.dma_start(x3[lb][:, 0], x_ext[n, lb, :, 0])
                        nc.sync.dma_start(x3[lb][:, 1], x_ext[n, lb, :, 1])
                else:
                    # Later images also ride sync (program order naturally
                    # defers them past the critical head); keeping them off
                    # the scalar ring avoids HBM contention with qt while the
                    # first matmuls' inputs stream in.
                    for lb in range(2):
                        nc.sync.dma_start(x3[lb][:], x_ext[n, lb])
                # tT[r_in, rc, vi_in, vj] per l-block
                tT = tpool.tile([C, 2, 2, 16, 32], mm_dtype)
                # rc-outer: the first two groups only need r2t[0], giving the
                # r2t[1] DMA until ~2 group-times after the first matmul.
                for rc in range(2):   # rank tiles of 128
                    for lb in range(2):   # l-blocks of 512 positions
                        pt = ptpool.tile([128, 16, 32], DT)
                        for k, win in enumerate(WIN_SEQ):
                            i, j = divmod(win, 4)
                            rhs = x3[lb][:, i & 1, j & 1,
                                         (i >> 1) : (i >> 1) + 16,
                                         (j >> 1) : (j >> 1) + 32]
                            nc.tensor.matmul(
                                pt[:],
                                r2t[rc][:, k, :],
                                rhs,
                                start=(k == 0),
                                stop=(k == NWIN - 1),
                            )
                        nc.vector.tensor_copy(tT[:, rc, lb], pt[:])
                for lb in range(2):
                    osb = opool.tile([C, 2, 2, 16, 32], out_dt, tag="osb")
                    last = n == n_per_core - 1
                    for mt in range(4):   # output row tiles: m = mt*128 + oc
                        py = pypool.tile([128, 16, 32], DT)
                        for rc in range(2):
                            nc.tensor.matmul(
                                py[:],
                                qt[:, rc, mt * 128 : (mt + 1) * 128],
                                tT[:, rc, lb],
                                start=(rc == 0),
                                stop=(rc == 1),
                            )
                        top, left = divmod(mt, 2)
                        # last image: copies split across vector+scalar(ACT)
                        # and each quarter DMA'd as soon as it's copied,
                        # issues alternating scalar/sync — those engines are
                        # idle by then and this minimizes the exposed tail.
                        if last and mt >= 2:
                            nc.scalar.copy(osb[:, top, left], py[:])
                        else:
                            nc.vector.tensor_copy(osb[:, top, left], py[:])
                        if last:
                            deng = nc.scalar if mt % 2 == 0 else nc.sync
                            deng.dma_start(y_ext[n, lb, :, top, left], osb[:, top, left])
                    if not last:
                        for top in range(2):
                            nc.scalar.dma_start(y_ext[n, lb, :, top], osb[:, top])
    nc.finalize()
    return nc


NWARM_WINO = 16

# --- Winograd F(2x2,2x2) variant -------------------------------------------
# mm1 (t = R * windows) is 4 parity-plane 2x2/stride-1 correlations folded
# into one 2x2 correlation with 512 in-channels (c x parity). Winograd with
# 1D transforms  Gg = [g0, g0+g1, g1],  B^T d = [d0-d1, d1, d2-d1],
# A^T = [[1,1,0],[0,1,1]]  computes each 2x2 output tile with 9 products
# instead of 16, cutting mm1 PE rows by 9/16. The input transform runs on
# the host (x~ is 2x the raw input bytes, still well under the PE time as
# bf16); the 9->4 inverse transform is 6 tensor_adds per (image, rank-half)
# on the otherwise-idle gpsimd engine. mm2 (out = Q @ t) is unchanged.
#
# Layout: tiles ti,tj in [0,16); output position vi = 2*ti + a, vj = 2*tj+b;
# l-block lb = ti>>3, u = ti & 7. t/y free layout is (a, u, tj, b) so every
# transform write is a plain affine view (no stride-2 slices needed); the
# host unshard reassembles h = 32*lb + 4*u + 2*a + top, w = 4*tj + 2*b + left.


def build_nc_wino(n_per_core=N_PER_CORE, mm_dtype=MM_DT):
    out_dt = mybir.dt.bfloat16
    nc = bacc.Bacc()
    xt_ext = nc.declare_dram_parameter("x", [n_per_core, C, 9, 4, 16, 16], mm_dtype, isOutput=False)
    rt_ext = nc.declare_dram_parameter("rt", [C, 9, 4, 2, 128], mm_dtype, isOutput=False)
    q_ext = nc.declare_dram_parameter("qt", [C, 2, MOUT], mm_dtype, isOutput=False)
    y_ext = nc.declare_dram_parameter("y", [n_per_core, 2, C, 2, 2, 512], out_dt, isOutput=True)

    with tile.TileContext(nc) as tc:
        with (
            tc.tile_pool(name="weights", bufs=1) as wpool,
            tc.tile_pool(name="xw", bufs=4) as xpool,
            tc.tile_pool(name="tw", bufs=2) as twpool,
            tc.tile_pool(name="uw", bufs=2) as upool,
            tc.tile_pool(name="tt", bufs=2) as tpool,
            tc.tile_pool(name="osb", bufs=4) as opool,
            tc.tile_pool(name="pt", bufs=4, space="PSUM") as ptpool,
            tc.tile_pool(name="py", bufs=4, space="PSUM") as pypool,
        ):
            warm = wpool.tile([C, 256], mm_dtype, name="warm")
            nc.gpsimd.memset(warm[:], 0.0)
            wpt = ptpool.tile([128, 2, 16, 16], DT, tag="pt", name="wpt")
            for _ in range(NWARM_WINO):
                nc.tensor.matmul(wpt[:, 0], warm[:, :128], warm[:], start=True, stop=True)

            rt = wpool.tile([C, 9, 4, 2, 128], mm_dtype, name="rt")
            qt = wpool.tile([C, 2, MOUT], mm_dtype, name="qt")
            # weights stream on sync, finest-first; x images alternate rings.
            nc.sync.dma_start(rt[:, 0:1], rt_ext[:, 0:1])
            nc.sync.dma_start(rt[:, 1:3], rt_ext[:, 1:3])
            nc.sync.dma_start(rt[:, 3:6], rt_ext[:, 3:6])
            nc.sync.dma_start(rt[:, 6:9], rt_ext[:, 6:9])

            for n in range(n_per_core):
                xt = xpool.tile([C, 9, 4, 16, 16], mm_dtype, tag="xw", name=f"xt{n}")
                if n == 0:
                    nc.scalar.dma_start(xt[:, 0:1], xt_ext[n][:, 0:1])
                    nc.scalar.dma_start(xt[:, 1:3], xt_ext[n][:, 1:3])
                    nc.scalar.dma_start(xt[:, 3:6], xt_ext[n][:, 3:6])
                    nc.scalar.dma_start(xt[:, 6:9], xt_ext[n][:, 6:9])
                    nc.scalar.dma_start(qt[:], q_ext[:])
                else:
                    eng = nc.sync if n % 2 else nc.scalar
                    eng.dma_start(xt[:, 0:3], xt_ext[n][:, 0:3])
                    eng.dma_start(xt[:, 3:9], xt_ext[n][:, 3:9])

                tT = tpool.tile([C, 2, 2, 2, 8, 16, 2], mm_dtype, tag="tt", name=f"tT{n}")
                for mt in range(2):   # rank-halves of t
                    T = twpool.tile([C, 3, 3, 16, 16], mm_dtype, tag="tw", name=f"T{n}_{mt}")
                    for kr in range(3):
                        for kc in range(3):
                            k = kr * 3 + kc
                            pt = ptpool.tile([128, 2, 16, 16], DT, tag="pt", name="pt")
                            for pp in range(4):
                                nc.tensor.matmul(
                                    pt[:, 0],
                                    rt[:, k, pp, mt, :],
                                    xt[:, k, pp],
                                    start=(pp == 0),
                                    stop=(pp == 3),
                                )
                            # T[kc-major, kr] so the inverse's kc-contractions
                            # read contiguous views.
                            nc.vector.tensor_copy(T[:, kc, kr], pt[:, 0])
                    # inverse transform: kc pass then kr pass, all affine views
                    U = upool.tile([C, 3, 16, 16, 2], mm_dtype, tag="uw", name=f"U{n}_{mt}")
                    nc.gpsimd.tensor_add(U[:, :, :, :, 0], T[:, 0], T[:, 1])
                    nc.gpsimd.tensor_add(U[:, :, :, :, 1], T[:, 1], T[:, 2])
                    for a in range(2):
                        for lb in range(2):
                            nc.gpsimd.tensor_add(
                                tT[:, mt, lb, a],
                                U[:, a, 8 * lb : 8 * lb + 8],
                                U[:, a + 1, 8 * lb : 8 * lb + 8],
                            )
                for lb in range(2):
                    osb = opool.tile([C, 2, 2, 512], out_dt, tag="osb", name="osb")
                    last = n == n_per_core - 1
                    for mt in range(4):   # output row tiles: m = mt*128 + oc
                        py = pypool.tile([128, 512], DT, tag="py", name="py")
                        for rc in range(2):
                            nc.tensor.matmul(
                                py[:],
                                qt[:, rc, mt * 128 : (mt + 1) * 128],
                                tT[:, rc, lb],
                                start=(rc == 0),
                                stop=(rc == 1),
                            )
                        top, left = divmod(mt, 2)
                        if last and mt >= 2:
                            nc.scalar.copy(osb[:, top, left], py[:])
                        else:
                            nc.vector.tensor_copy(osb[:, top, left], py[:])
                        if last:
                            deng = nc.scalar if mt % 2 == 0 else nc.sync
                            deng.dma_start(y_ext[n, lb, :, top, left], osb[:, top, left])
                    if not last:
                        for top in range(2):
                            nc.scalar.dma_start(y_ext[n, lb, :, top], osb[:, top])
    nc.finalize()
    return nc


_BW = np.array([[1, -1, 0], [0, 1, 0], [0, -1, 1]], np.float32)
_GW = np.array([[1, 0], [1, 1], [0, 1]], np.float32)


def make_host_inputs_wino(x, Q, R, np_dtype=None):
    """Full inputs -> (x~ winograd tiles, R~ winograd weights, qt)."""
    if np_dtype is None:
        import ml_dtypes
        np_dtype = ml_dtypes.bfloat16
    x = np.asarray(x, dtype=np.float32)
    Q = np.asarray(Q, dtype=np.float32)
    R = np.asarray(R, dtype=np.float32)
    n = x.shape[0]
    xpad = np.zeros((n, C, 66, 66), np.float32)
    xpad[:, :, 1 : 1 + H, 1 : 1 + W] = x
    s2d = xpad.reshape(n, C, 33, 2, 33, 2).transpose(0, 1, 3, 5, 2, 4)  # [n,C,pi,pj,33,33]
    d = np.empty((n, C, 2, 2, 3, 3, 16, 16), np.float32)
    for kr in range(3):
        for kc in range(3):
            d[:, :, :, :, kr, kc] = s2d[:, :, :, :, kr : kr + 32 : 2, kc : kc + 32 : 2]
    xt = np.einsum('ka,ncpqabij,lb->ncklpqij', _BW, d, _BW)   # [n,C,kr,kc,pi,pj,16,16]
    xt = np.ascontiguousarray(xt.reshape(n, C, 9, 4, 16, 16)).astype(np_dtype)
    # R4[r, c, pi, pj, a, b] = R[r, c*16 + (2a+pi)*4 + (2b+pj)]
    R4 = R.reshape(RANK, C, 2, 2, 2, 2).transpose(0, 1, 3, 5, 2, 4)
    Rt = np.einsum('ka,rcpqab,lb->cklpqr', _GW, R4, _GW)      # [C,kr,kc,pi,pj,r]
    rt = np.ascontiguousarray(Rt.reshape(C, 9, 4, 2, 128)).astype(np_dtype)
    qt = np.ascontiguousarray(Q.reshape(MOUT, 2, 128).transpose(2, 1, 0)).astype(np_dtype)
    return xt, rt, qt


def unshard_output_wino(ys):
    """Per-core [npc, 2, C, 2, 2, 512] (a,u,tj,b)-layout -> [N, C, 64, 64]."""
    y = np.concatenate([np.asarray(v, dtype=np.float32) for v in ys], axis=0)
    n = y.shape[0]
    y = y.reshape(n, 2, C, 2, 2, 2, 8, 16, 2)  # [n,lb,c,top,left,a,u,tj,b]
    # h = 32*lb + 4*u + 2*a + top ; w = 4*tj + 2*b + left
    y = y.transpose(0, 2, 1, 6, 5, 3, 7, 8, 4).reshape(n, C, 64, 64)
    return np.ascontiguousarray(y)


def make_host_inputs(x, Q, R, np_dtype=None):
    """Full inputs -> (x3 chunks, r2t halves, qt) host arrays."""
    if np_dtype is None:
        import ml_dtypes
        np_dtype = ml_dtypes.bfloat16
    x = np.asarray(x, dtype=np.float32)
    Q = np.asarray(Q, dtype=np.float32)
    R = np.asarray(R, dtype=np.float32)
    n = x.shape[0]
    xpad = np.zeros((n, C, 66, 66), np.float32)
    xpad[:, :, 1 : 1 + H, 1 : 1 + W] = x
    # space-to-depth: s2d[n, c, pi, pj, hi, wi] = xpad[n, c, 2hi+pi, 2wi+pj]
    s2d = xpad.reshape(n, C, 33, 2, 33, 2).transpose(0, 1, 3, 5, 2, 4)
    # l-block chunks with duplicated boundary row hi=16:
    # x3[n, lb, c, pi, pj, h, w] = s2d[n, c, pi, pj, 16*lb+h, w]
    x3 = np.empty((n, 2, C, 2, 2, 17, 33), np.float32)
    x3[:, 0] = s2d[:, :, :, :, 0:17]
    x3[:, 1] = s2d[:, :, :, :, 16:33]
    x3 = np.ascontiguousarray(x3).astype(np_dtype)
    # permute R columns from (c*16+win) to (win*128+c), split by rank half
    R2 = R.reshape(RANK, C, NWIN).transpose(0, 2, 1).reshape(RANK, C * NWIN)
    r2t = R2.reshape(2, 128, NWIN, C).transpose(0, 3, 2, 1)  # [rc, c, win, r_in]
    r2t = r2t[:, :, WIN_SEQ, :]  # win axis in device processing order
    r2t = np.ascontiguousarray(r2t).astype(np_dtype)
    qt = np.ascontiguousarray(Q.reshape(MOUT, 2, 128).transpose(2, 1, 0)).astype(np_dtype)
    return x3, r2t, qt


def unshard_output(ys):
    """Per-core [npc, 2, C, 2, 2, 16, 32] l-block parity planes -> [N, C, 64, 64]."""
    y5 = np.concatenate([np.asarray(y, dtype=np.float32) for y in ys], axis=0)
    n = y5.shape[0]
    # h = 32*lb + 2*vi_in + top ; w = 2*vj + left
    y = y5.transpose(0, 2, 1, 5, 3, 6, 4).reshape(n, C, 64, 64)
    return np.ascontiguousarray(y)


_NC_CACHE = {}


def kernel(x, Q, R):
    x3, r2t, qt = make_host_inputs(x, Q, R)
    n = x3.shape[0]
    assert n == N_CORES * N_PER_CORE
    if "nc" not in _NC_CACHE:
        _NC_CACHE["nc"] = build_nc()
    nc = _NC_CACHE["nc"]
    in_maps = [
        {
            "x": np.ascontiguousarray(x3[i * N_PER_CORE : (i + 1) * N_PER_CORE]),
            "r2t": r2t,
            "qt": qt,
        }
        for i in range(N_CORES)
    ]
    res = run_bass_kernel_spmd(nc, in_maps, list(range(N_CORES)))
    return unshard_output([res.results[i]["y"] for i in range(N_CORES)])


# revision 12
# speedup vs baseline: 1.2298x; 1.2298x over previous
# Trainium2 Bass kernel for nn_Conv2dSDK_QR: low-rank (Q @ R) factorized
# stride-1 3x3 conv expressed as two matmuls over 4x4/stride-2 windows.
#
# Math (per image, validated vs reference):
#   xp = zero-pad(x, 1)                              [128, 66, 66]
#   flatT[win*128+c, vi*32+vj] = xp[c, i+2vi, j+2vj] (win = i*4+j)
#   tT = R2 @ flatT                                  [256, 1024]
#   yT = Q @ tT                                      [512, 1024]
#   out[oc, 2vi+top, 2vj+left] = yT[(top*2+left)*128+oc, vi*32+vj]
# where R2 is R with columns permuted from (c*16+win) to (win*128+c)
# ordering, so each win-chunk of flatT is just a strided view of xp.
#
# Device layouts (host pre/post-processed so every PE stream and every DMA
# is contiguous):
#   space-to-depth: s2d[c, pi, pj, hi, wi] = xp[c, 2hi+pi, 2wi+pj] (66=2x33)
#   x3[lb][c, pi, pj, h, w] = s2d[c, pi, pj, 16*lb + h, w], h in [0,17)
#     (l-block chunks, boundary row hi=16 duplicated) -> window (i,j) of
#     l-block lb is the contiguous-inner view
#     x3[lb][:, i&1, j&1, (i>>1):(i>>1)+16, (j>>1):(j>>1)+32]
#   y per l-block: y3[lb][oc, top, left, vi_in, vj] = out[oc, 2(16lb+vi_in)+top, 2vj+left]
#
# Sharding: data-parallel over batch, 4 images per core across 8 cores.
#
# Schedule notes (from ntff profile analysis):
# - The framework preamble ends ~7.3us; DMA packets start landing ~1.5us
#   after the first dma_start issue. The tensor engine runs at a reduced
#   p-state for its first ~3us of continuous execution, so a string of
#   warmup matmuls on a memset tile is issued first: they execute during
#   the input-DMA head and bring the PE to full clock before real work.
# - The sync ring carries everything the first two images' matmuls gate
#   on, interleaved finest-first: [r2t0 w0-3][x0 lb0 pi0][r2t0 w4-11]
#   [x0 lb0 pi1][r2t0 w12-15][x0 lb1][r2t1][x1]. The scalar ring carries
#   qt + images 2,3 + all output, so the slow gpsimd SWDGE ring is unused.
# - bf16 end-to-end (rel err ~3e-3, gate 2e-2): halves DMA bytes and DVE
#   copy traffic vs f32; matmul rate on the PE is 1 row/cycle either way.

import numpy as np

import concourse.bacc as bacc
import concourse.bass as bass
import concourse.mybir as mybir
import concourse.tile as tile
from concourse.bass_utils import run_bass_kernel_spmd

N_CORES = 8
N_PER_CORE = 4
C = 128          # channels (= partition dim)
H = W = 64
RANK = 256
MOUT = 512       # 4 placements * 128 out channels
NWIN = 16        # 4x4 window positions
DT = mybir.dt.float32
MM_DT = mybir.dt.bfloat16
NWARM = 12       # p-state warmup matmuls (N=256 each) before real work
# Window processing order: even-i (pi=0) windows first, so the first matmul
# group can start as soon as the pi=0 half of the image chunk and the first
# weight chunk have landed. The r2t win axis is host-permuted to this order.
WIN_SEQ = [0, 1, 2, 3, 8, 9, 10, 11, 4, 5, 6, 7, 12, 13, 14, 15]


def _out_dt(mm_dtype):
    # bf16 output halves the write traffic; keep f32 output for the f32r
    # variant so it stays the high-precision reference configuration.
    return mybir.dt.bfloat16 if mm_dtype == mybir.dt.bfloat16 else DT


def build_nc(n_per_core=N_PER_CORE, mm_dtype=MM_DT):
    out_dt = _out_dt(mm_dtype)
    nc = bacc.Bacc()
    x_ext = nc.declare_dram_parameter("x", [n_per_core, 2, C, 2, 2, 17, 33], mm_dtype, isOutput=False)
    r_ext = nc.declare_dram_parameter("r2t", [2, C, NWIN, 128], mm_dtype, isOutput=False)
    q_ext = nc.declare_dram_parameter("qt", [C, 2, MOUT], mm_dtype, isOutput=False)
    y_ext = nc.declare_dram_parameter("y", [n_per_core, 2, C, 2, 2, 16, 32], out_dt, isOutput=True)

    with tile.TileContext(nc) as tc:
        with (
            tc.tile_pool(name="weights", bufs=1) as wpool,
            tc.tile_pool(name="xp", bufs=8) as xpool,
            tc.tile_pool(name="tt", bufs=2) as tpool,
            tc.tile_pool(name="osb", bufs=4) as opool,
            tc.tile_pool(name="pt", bufs=4, space="PSUM") as ptpool,
            tc.tile_pool(name="py", bufs=4, space="PSUM") as pypool,
        ):
            # PE p-state warmup: matmuls on a zeroed tile, results discarded.
            # These run during the input-DMA head so the first real matmul
            # already executes at full clock.
            warm = wpool.tile([C, 256], mm_dtype, name="warm")
            nc.gpsimd.memset(warm[:], 0.0)
            wpt = ptpool.tile([128, 16, 32], DT, tag="pt", name="wpt")
            for _ in range(NWARM):
                nc.tensor.matmul(wpt[:, :8], warm[:, :128], warm[:], start=True, stop=True)

            # r2t[rc][c, win, r_in] = R2[rc*128+r_in, win*128+c]; the win axis
            # is host-permuted to WIN_SEQ order, so [:, :4] is the first four
            # windows processed.
            r2t = [wpool.tile([C, NWIN, 128], mm_dtype, tag=f"r2t{rc}", name=f"r2t{rc}") for rc in range(2)]
            # qt[r_in, rc, m] = Q[m, rc*128+r_in]  (lhsT chunks for matmul 2)
            qt = wpool.tile([C, 2, MOUT], mm_dtype)
            nc.sync.dma_start(r2t[0][:, :4], r_ext[0][:, :4])
            # qt is only needed by the first matmul-2 group (~16us after the
            # first matmul); the scalar HWDGE ring has it long before then.
            nc.scalar.dma_start(qt[:], q_ext[:])

            for n in range(n_per_core):
                x3 = [xpool.tile([C, 2, 2, 17, 33], mm_dtype, tag="x3", name=f"x3_{n}_{lb}") for lb in range(2)]
                if n == 0:
                    # Interleave weight chunks with the pi-plane halves the
                    # first matmuls consume, finest chunks first, all on the
                    # early-starting sync ring.
                    nc.sync.dma_start(x3[0][:, 0], x_ext[0, 0, :, 0])
                    nc.sync.dma_start(r2t[0][:, 4:12], r_ext[0][:, 4:12])
                    nc.sync.dma_start(x3[0][:, 1], x_ext[0, 0, :, 1])
                    nc.sync.dma_start(r2t[0][:, 12:], r_ext[0][:, 12:])
                    nc.sync.dma_start(x3[1][:, 0], x_ext[0, 1, :, 0])
                    nc.sync.dma_start(x3[1][:, 1], x_ext[0, 1, :, 1])
                    nc.sync.dma_start(r2t[1][:], r_ext[1])
                elif n == 1:
                    for lb in range(2):
                        nc.sync.dma_start(x3[lb][:, 0], x_ext[n, lb, :, 0])
                        nc.sync.dma_start(x3[lb][:, 1], x_ext[n, lb, :, 1])
                else:
                    # Later images also ride sync (program order naturally
                    # defers them past the critical head); keeping them off
                    # the scalar ring avoids HBM contention with qt while the
                    # first matmuls' inputs stream in.
                    for lb in range(2):
                        nc.sync.dma_start(x3[lb][:], x_ext[n, lb])
                # tT[r_in, rc, vi_in, vj] per l-block
                tT = tpool.tile([C, 2, 2, 16, 32], mm_dtype)
                # rc-outer: the first two groups only need r2t[0], giving the
                # r2t[1] DMA until ~2 group-times after the first matmul.
                for rc in range(2):   # rank tiles of 128
                    for lb in range(2):   # l-blocks of 512 positions
                        pt = ptpool.tile([128, 16, 32], DT)
                        for k, win in enumerate(WIN_SEQ):
                            i, j = divmod(win, 4)
                            rhs = x3[lb][:, i & 1, j & 1,
                                         (i >> 1) : (i >> 1) + 16,
                                         (j >> 1) : (j >> 1) + 32]
                            nc.tensor.matmul(
                                pt[:],
                                r2t[rc][:, k, :],
                                rhs,
                                start=(k == 0),
                                stop=(k == NWIN - 1),
                            )
                        nc.vector.tensor_copy(tT[:, rc, lb], pt[:])
                for lb in range(2):
                    osb = opool.tile([C, 2, 2, 16, 32], out_dt, tag="osb")
                    last = n == n_per_core - 1
                    for mt in range(4):   # output row tiles: m = mt*128 + oc
                        py = pypool.tile([128, 16, 32], DT)
                        for rc in range(2):
                            nc.tensor.matmul(
                                py[:],
                                qt[:, rc, mt * 128 : (mt + 1) * 128],
                                tT[:, rc, lb],
                                start=(rc == 0),
                                stop=(rc == 1),
                            )
                        top, left = divmod(mt, 2)
                        # last image: copies split across vector+scalar(ACT)
                        # and each quarter DMA'd as soon as it's copied,
                        # issues alternating scalar/sync — those engines are
                        # idle by then and this minimizes the exposed tail.
                        if last and mt >= 2:
                            nc.scalar.copy(osb[:, top, left], py[:])
                        else:
                            nc.vector.tensor_copy(osb[:, top, left], py[:])
                        if last:
                            deng = nc.scalar if mt % 2 == 0 else nc.sync
                            deng.dma_start(y_ext[n, lb, :, top, left], osb[:, top, left])
                    if not last:
                        for top in range(2):
                            nc.scalar.dma_start(y_ext[n, lb, :, top], osb[:, top])
    nc.finalize()
    return nc


NWARM_WINO = 16

# --- Winograd F(2x2,2x2) variant -------------------------------------------
# mm1 (t = R * windows) is 4 parity-plane 2x2/stride-1 correlations folded
# into one 2x2 correlation with 512 in-channels (c x parity). Winograd with
# 1D transforms  Gg = [g0, g0+g1, g1],  B^T d = [d0-d1, d1, d2-d1],
# A^T = [[1,1,0],[0,1,1]]  computes each 2x2 output tile with 9 products
# instead of 16, cutting mm1 PE rows by 9/16. The input transform runs on
# the host (x~ is 2x the raw input bytes, still well under the PE time as
# bf16); the 9->4 inverse transform is 6 tensor_adds per (image, rank-half)
# on the otherwise-idle gpsimd engine. mm2 (out = Q @ t) is unchanged.
#
# Layout: tiles ti,tj in [0,16); output position vi = 2*ti + a, vj = 2*tj+b;
# l-block lb = ti>>3, u = ti & 7. t/y free layout is (a, u, tj, b) so every
# transform write is a plain affine view (no stride-2 slices needed); the
# host unshard reassembles h = 32*lb + 4*u + 2*a + top, w = 4*tj + 2*b + left.


def build_nc_wino(n_per_core=N_PER_CORE, mm_dtype=MM_DT):
    out_dt = mybir.dt.bfloat16
    nc = bacc.Bacc()
    xt_ext = nc.declare_dram_parameter("x", [n_per_core, C, 9, 4, 16, 16], mm_dtype, isOutput=False)
    rt_ext = nc.declare_dram_parameter("rt", [C, 9, 4, 2, 128], mm_dtype, isOutput=False)
    q_ext = nc.declare_dram_parameter("qt", [C, 2, MOUT], mm_dtype, isOutput=False)
    y_ext = nc.declare_dram_parameter("y", [n_per_core, 2, C, 2, 2, 512], out_dt, isOutput=True)

    with tile.TileContext(nc) as tc:
        with (
            tc.tile_pool(name="weights", bufs=1) as wpool,
            tc.tile_pool(name="xw", bufs=4) as xpool,
            tc.tile_pool(name="tw", bufs=2) as twpool,
            tc.tile_pool(name="uw", bufs=2) as upool,
            tc.tile_pool(name="tt", bufs=2) as tpool,
            tc.tile_pool(name="osb", bufs=4) as opool,
            tc.tile_pool(name="pt", bufs=4, space="PSUM") as ptpool,
            tc.tile_pool(name="py", bufs=4, space="PSUM") as pypool,
        ):
            warm = wpool.tile([C, 256], mm_dtype, name="warm")
            nc.gpsimd.memset(warm[:], 0.0)
            wpt = ptpool.tile([128, 2, 16, 16], DT, tag="pt", name="wpt")
            for _ in range(NWARM_WINO):
                nc.tensor.matmul(wpt[:, 0], warm[:, :128], warm[:], start=True, stop=True)

            rt = wpool.tile([C, 9, 4, 2, 128], mm_dtype, name="rt")
            qt = wpool.tile([C, 2, MOUT], mm_dtype, name="qt")
            # weights stream on sync, finest-first; x images alternate rings.
            nc.sync.dma_start(rt[:, 0:1], rt_ext[:, 0:1])
            nc.sync.dma_start(rt[:, 1:3], rt_ext[:, 1:3])
            nc.sync.dma_start(rt[:, 3:6], rt_ext[:, 3:6])
            nc.sync.dma_start(rt[:, 6:9], rt_ext[:, 6:9])

            def emit_mm2(n, tT):
                # second matmul out = Q @ t, plus output staging/DMA
                last = n == n_per_core - 1
                for lb in range(2):
                    osb = opool.tile([C, 2, 2, 512], out_dt, tag="osb", name="osb")
                    for mt in range(4):   # output row tiles: m = mt*128 + oc
                        py = pypool.tile([128, 512], DT, tag="py", name="py")
                        for rc in range(2):
                            nc.tensor.matmul(
                                py[:],
                                qt[:, rc, mt * 128 : (mt + 1) * 128],
                                tT[:, rc, lb],
                                start=(rc == 0),
                                stop=(rc == 1),
                            )
                        top, left = divmod(mt, 2)
                        # psum->sbuf copies ride scalar(ACT), keeping vector
                        # free for the winograd-domain copies; the last
                        # image's first two quarters go to vector to cut the
                        # exposed tail.
                        if last and mt < 2:
                            nc.vector.tensor_copy(osb[:, top, left], py[:])
                        else:
                            nc.scalar.copy(osb[:, top, left], py[:])
                        if last:
                            deng = nc.scalar if mt % 2 == 0 else nc.sync
                            deng.dma_start(y_ext[n, lb, :, top, left], osb[:, top, left])
                    if not last:
                        for top in range(2):
                            nc.scalar.dma_start(y_ext[n, lb, :, top], osb[:, top])

            prev = None
            for n in range(n_per_core):
                xt = xpool.tile([C, 9, 4, 16, 16], mm_dtype, tag="xw", name=f"xt{n}")
                if n == 0:
                    nc.scalar.dma_start(xt[:, 0:1], xt_ext[n][:, 0:1])
                    nc.scalar.dma_start(xt[:, 1:3], xt_ext[n][:, 1:3])
                    nc.scalar.dma_start(xt[:, 3:6], xt_ext[n][:, 3:6])
                    nc.scalar.dma_start(xt[:, 6:9], xt_ext[n][:, 6:9])
                    nc.scalar.dma_start(qt[:], q_ext[:])
                else:
                    eng = nc.sync if n % 2 else nc.scalar
                    eng.dma_start(xt[:, 0:3], xt_ext[n][:, 0:3])
                    eng.dma_start(xt[:, 3:9], xt_ext[n][:, 3:9])

                tT = tpool.tile([C, 2, 2, 2, 8, 16, 2], mm_dtype, tag="tt", name=f"tT{n}")
                for mt in range(2):   # rank-halves of t
                    # T flat over k = kr*3+kc; chains pair up (k, k+1) in one
                    # psum tile so one copy drains two chains.
                    T = twpool.tile([C, 9, 16, 16], mm_dtype, tag="tw", name=f"T{n}_{mt}")
                    for kp in range(5):
                        ks = [2 * kp, 2 * kp + 1] if kp < 4 else [8]
                        pt = ptpool.tile([128, 2, 16, 16], DT, tag="pt", name="pt")
                        for half, k in enumerate(ks):
                            for pp in range(4):
                                nc.tensor.matmul(
                                    pt[:, half],
                                    rt[:, k, pp, mt, :],
                                    xt[:, k, pp],
                                    start=(pp == 0),
                                    stop=(pp == 3),
                                )
                        nc.vector.tensor_copy(T[:, ks[0] : ks[-1] + 1], pt[:, : len(ks)])
                    # inverse transform on gpsimd: kc pass (stride-3 views)
                    # then kr pass, all affine
                    U = upool.tile([C, 3, 16, 16, 2], mm_dtype, tag="uw", name=f"U{n}_{mt}")
                    nc.gpsimd.tensor_add(U[:, :, :, :, 0], T[:, 0::3], T[:, 1::3])
                    nc.gpsimd.tensor_add(U[:, :, :, :, 1], T[:, 1::3], T[:, 2::3])
                    for a in range(2):
                        for lb in range(2):
                            nc.gpsimd.tensor_add(
                                tT[:, mt, lb, a],
                                U[:, a, 8 * lb : 8 * lb + 8],
                                U[:, a + 1, 8 * lb : 8 * lb + 8],
                            )
                # software pipeline: image n-1's mm2 sits behind image n's mm1
                # in the tensor queue, so the PE never waits for the inverse
                # transform chain.
                if prev is not None:
                    emit_mm2(*prev)
                prev = (n, tT)
            emit_mm2(*prev)
    nc.finalize()
    return nc


_BW = np.array([[1, -1, 0], [0, 1, 0], [0, -1, 1]], np.float32)
_GW = np.array([[1, 0], [1, 1], [0, 1]], np.float32)


def make_host_inputs_wino(x, Q, R, np_dtype=None):
    """Full inputs -> (x~ winograd tiles, R~ winograd weights, qt)."""
    if np_dtype is None:
        import ml_dtypes
        np_dtype = ml_dtypes.bfloat16
    x = np.asarray(x, dtype=np.float32)
    Q = np.asarray(Q, dtype=np.float32)
    R = np.asarray(R, dtype=np.float32)
    n = x.shape[0]
    xpad = np.zeros((n, C, 66, 66), np.float32)
    xpad[:, :, 1 : 1 + H, 1 : 1 + W] = x
    s2d = xpad.reshape(n, C, 33, 2, 33, 2).transpose(0, 1, 3, 5, 2, 4)  # [n,C,pi,pj,33,33]
    d = np.empty((n, C, 2, 2, 3, 3, 16, 16), np.float32)
    for kr in range(3):
        for kc in range(3):
            d[:, :, :, :, kr, kc] = s2d[:, :, :, :, kr : kr + 32 : 2, kc : kc + 32 : 2]
    xt = np.einsum('ka,ncpqabij,lb->ncklpqij', _BW, d, _BW)   # [n,C,kr,kc,pi,pj,16,16]
    xt = np.ascontiguousarray(xt.reshape(n, C, 9, 4, 16, 16)).astype(np_dtype)
    # R4[r, c, pi, pj, a, b] = R[r, c*16 + (2a+pi)*4 + (2b+pj)]
    R4 = R.reshape(RANK, C, 2, 2, 2, 2).transpose(0, 1, 3, 5, 2, 4)
    Rt = np.einsum('ka,rcpqab,lb->cklpqr', _GW, R4, _GW)      # [C,kr,kc,pi,pj,r]
    rt = np.ascontiguousarray(Rt.reshape(C, 9, 4, 2, 128)).astype(np_dtype)
    qt = np.ascontiguousarray(Q.reshape(MOUT, 2, 128).transpose(2, 1, 0)).astype(np_dtype)
    return xt, rt, qt


def unshard_output_wino(ys):
    """Per-core [npc, 2, C, 2, 2, 512] (a,u,tj,b)-layout -> [N, C, 64, 64]."""
    y = np.concatenate([np.asarray(v, dtype=np.float32) for v in ys], axis=0)
    n = y.shape[0]
    y = y.reshape(n, 2, C, 2, 2, 2, 8, 16, 2)  # [n,lb,c,top,left,a,u,tj,b]
    # h = 32*lb + 4*u + 2*a + top ; w = 4*tj + 2*b + left
    y = y.transpose(0, 2, 1, 6, 5, 3, 7, 8, 4).reshape(n, C, 64, 64)
    return np.ascontiguousarray(y)


def make_host_inputs(x, Q, R, np_dtype=None):
    """Full inputs -> (x3 chunks, r2t halves, qt) host arrays."""
    if np_dtype is None:
        import ml_dtypes
        np_dtype = ml_dtypes.bfloat16
    x = np.asarray(x, dtype=np.float32)
    Q = np.asarray(Q, dtype=np.float32)
    R = np.asarray(R, dtype=np.float32)
    n = x.shape[0]
    xpad = np.zeros((n, C, 66, 66), np.float32)
    xpad[:, :, 1 : 1 + H, 1 : 1 + W] = x
    # space-to-depth: s2d[n, c, pi, pj, hi, wi] = xpad[n, c, 2hi+pi, 2wi+pj]
    s2d = xpad.reshape(n, C, 33, 2, 33, 2).transpose(0, 1, 3, 5, 2, 4)
    # l-block chunks with duplicated boundary row hi=16:
    # x3[n, lb, c, pi, pj, h, w] = s2d[n, c, pi, pj, 16*lb+h, w]
    x3 = np.empty((n, 2, C, 2, 2, 17, 33), np.float32)
    x3[:, 0] = s2d[:, :, :, :, 0:17]
    x3[:, 1] = s2d[:, :, :, :, 16:33]
    x3 = np.ascontiguousarray(x3).astype(np_dtype)
    # permute R columns from (c*16+win) to (win*128+c), split by rank half
    R2 = R.reshape(RANK, C, NWIN).transpose(0, 2, 1).reshape(RANK, C * NWIN)
    r2t = R2.reshape(2, 128, NWIN, C).transpose(0, 3, 2, 1)  # [rc, c, win, r_in]
    r2t = r2t[:, :, WIN_SEQ, :]  # win axis in device processing order
    r2t = np.ascontiguousarray(r2t).astype(np_dtype)
    qt = np.ascontiguousarray(Q.reshape(MOUT, 2, 128).transpose(2, 1, 0)).astype(np_dtype)
    return x3, r2t, qt


def unshard_output(ys):
    """Per-core [npc, 2, C, 2, 2, 16, 32] l-block parity planes -> [N, C, 64, 64]."""
    y5 = np.concatenate([np.asarray(y, dtype=np.float32) for y in ys], axis=0)
    n = y5.shape[0]
    # h = 32*lb + 2*vi_in + top ; w = 2*vj + left
    y = y5.transpose(0, 2, 1, 5, 3, 6, 4).reshape(n, C, 64, 64)
    return np.ascontiguousarray(y)


_NC_CACHE = {}


def kernel(x, Q, R):
    x3, r2t, qt = make_host_inputs(x, Q, R)
    n = x3.shape[0]
    assert n == N_CORES * N_PER_CORE
    if "nc" not in _NC_CACHE:
        _NC_CACHE["nc"] = build_nc()
    nc = _NC_CACHE["nc"]
    in_maps = [
        {
            "x": np.ascontiguousarray(x3[i * N_PER_CORE : (i + 1) * N_PER_CORE]),
            "r2t": r2t,
            "qt": qt,
        }
        for i in range(N_CORES)
    ]
    res = run_bass_kernel_spmd(nc, in_maps, list(range(N_CORES)))
    return unshard_output([res.results[i]["y"] for i in range(N_CORES)])
